# revision 15
# baseline (speedup 1.0000x reference)
"""Multi-head attention with RoPE on 8 Trainium2 NeuronCores.

Problem: x[2,2048,1024] -> MHA(16 heads, hd=64, NeoX RoPE, non-causal) -> out.

Sharding: tensor-parallel over heads (each core owns 2 heads over the full
sequence). The output is resharded so each core owns 64-row blocks of every
512-row q-chunk of both batches (strided blocks), which lets the AllToAll
run in four quarter-sized pieces that overlap compute.

Host-side marshalling (inside kernel(), plain numpy): weights column-sliced
per core, x transposed per (batch, chunk) and cast to bf16, cos/sin
transposed+tiled — so the device kernel does no fp32 casting, no x
transposes, and loads everything with fast HWDGE DMAs.

Device pipeline (per core):
  - QKV projections + NeoX RoPE from the pre-transposed x^T,
  - flash-style attention with transposed scores [s_k, s_q]; the two local
    heads' scores land in one 4-bank PSUM group and one wide exp instruction
    per group covers both heads; softmax denominator from a fused ones
    column in V (exp bias keeps the fp32 range safe),
  - unnormalized numerator + sigma rows stored straight into half-batch
    AllToAll buffers (strided per-core 64-row blocks); each of the four
    A2As + normalization + Wo overlaps later attention chunks,
  - consumer-side normalization: 1/sigma partition-broadcast via a K=16
    selection-matrix matmul, multiplied into recv straight from PSUM,
  - emission order weaves next-batch prep and consumer work into the
    attention group loop so the PE instruction FIFO always has work.

All matmuls bf16 (fp32 PSUM); rel-err tolerance is 2e-2.
"""

import sys

sys.path.insert(0, "/opt/trn_rl_repo")

import ml_dtypes  # noqa: E402
import numpy as np  # noqa: E402

import concourse.bass as bass  # noqa: E402
import concourse.mybir as mybir  # noqa: E402
import concourse.tile as tile  # noqa: E402
from concourse.bass_utils import run_bass_kernel_spmd  # noqa: E402

N_CORES = 8
D = 1024
H = 16
HD = 64
HL = H // N_CORES  # 2 local heads per core
DL = HL * HD  # 128 local attn dims
EXP_SCALE = 0.125  # 1/sqrt(hd)
EXP_BIAS = -24.0  # exp(s/8 - 24): cancels in softmax, keeps fp32 range safe
GMAX = 2  # kt tiles per score group; one exp instruction covers both heads

F32 = mybir.dt.float32
BF16 = mybir.dt.bfloat16
BF16_NP = ml_dtypes.bfloat16


def _perm_matrix():
    """lhsT for the rotate_half matmul: qrot^T = lhsT.T @ q^T."""
    mt = np.zeros((DL, DL), dtype=np.float32)
    for o in (0, HD):
        for r in range(HD // 2):
            mt[o + r, o + r + HD // 2] = -1.0
            mt[o + r + HD // 2, o + r] = 1.0
    return np.ascontiguousarray(mt.T)


def _sel_matrix():
    """lhsT blocks for the 1/sigma partition-broadcast matmul.

    sigr partition layout is h*8+i (h = local head, i = source core);
    sel[r, i, p] = 1 iff r == (p//64)*8 + i.
    """
    sel = np.zeros((H, N_CORES, 128), dtype=np.float32)
    for i in range(N_CORES):
        sel[0 * N_CORES + i, i, 0:HD] = 1.0
        sel[1 * N_CORES + i, i, HD:128] = 1.0
    return sel


def split_excess_waits(nc, max_waits=1):
    """This container's walrus rejects >1 semaphore wait per instruction;
    split excess waits onto NoOp carriers on the same engine."""
    for bb in nc.m.functions[0].blocks:
        insts = bb.instructions
        idx = 0
        while idx < len(insts):
            ins = insts[idx]
            si = ins.sync_info
            if si is not None and si.on_wait and len(si.on_wait) > max_waits:
                ow = list(si.on_wait)
                si.on_wait = ow[-max_waits:]
                extra = ow[:-max_waits]
                k = 0
                while extra:
                    chunk, extra = extra[:max_waits], extra[max_waits:]
                    c = mybir.InstNoOp(name=f"{ins.name}-ws{k}", ins=[], outs=[])
                    c.engine = ins.engine
                    c.sync_info = mybir.SyncInfo(on_wait=chunk, on_update=[])
                    nc.register_instruction(c)
                    insts.insert(idx, c)
                    idx += 1
                    k += 1
            idx += 1


def build_nc(b=2, s=2048, chunk=512):
    kt = s // 128  # 16 score tiles along s_k per batch
    nch = s // chunk  # 4 s_q chunks per batch
    nhf = nch // 2  # 2 A2A halves per batch
    qb = chunk // N_CORES  # 64-row block per core per chunk
    rows_hf = 2 * qb  # 128 rows per core per A2A half
    shard = b * nch * qb  # 512 output rows per core
    ngr = kt // GMAX  # 8 score groups per chunk
    dt8 = D // 128
    assert nch == 4 and ngr == 8 and qb == 64

    nc = bass.Bass()
    xt = nc.declare_dram_parameter("xt", [b, nch, D, chunk], BF16, isOutput=False)
    csd = nc.declare_dram_parameter("csd", [128, s], BF16, isOutput=False)
    snd = nc.declare_dram_parameter("snd", [128, s], BF16, isOutput=False)
    wq = nc.declare_dram_parameter("wq", [128, dt8, DL], BF16, isOutput=False)
    wk = nc.declare_dram_parameter("wk", [128, dt8, DL], BF16, isOutput=False)
    wv = nc.declare_dram_parameter("wv", [128, dt8, DL], BF16, isOutput=False)
    wo = nc.declare_dram_parameter("wo", [128, dt8, D], BF16, isOutput=False)
    mperm = nc.declare_dram_parameter("mperm", [DL, DL], BF16, isOutput=False)
    identp = nc.declare_dram_parameter("ident", [128, 128], BF16, isOutput=False)
    selp = nc.declare_dram_parameter("sel", [H, N_CORES, 128], BF16, isOutput=False)
    out = nc.declare_dram_parameter("out", [shard, D], F32, isOutput=True)

    with tile.TileContext(nc) as tc:
        with (
            tc.tile_pool(name="dram", bufs=1, space="DRAM") as dram,
            tc.tile_pool(name="const", bufs=1) as cpool,
            tc.tile_pool(name="xt", bufs=1) as xtpool,
            tc.tile_pool(name="qkv", bufs=2) as qkvpool,
            tc.tile_pool(name="rope", bufs=2) as ropepool,
            tc.tile_pool(name="pt", bufs=4) as ptpool,
            tc.tile_pool(name="oh", bufs=4) as ohpool,
            tc.tile_pool(name="rc", bufs=2) as rcpool,
            tc.tile_pool(name="outp", bufs=2) as outpool,
            # PSUM (8 banks): "sc" score group [128, HL, GMAX, 512] fp32 = 4
            # banks (single-buffered; one wide exp per group drains it).
            # "pv" 2 x [65, 512] = 2 banks. "tp" general purpose = 2 banks.
            tc.tile_pool(name="psS", bufs=1, space="PSUM") as psS,
            tc.tile_pool(name="psB", bufs=2, space="PSUM") as psB,
            tc.tile_pool(name="psC", bufs=2, space="PSUM") as psC,
        ):
            # ---- staging: all loads are plain HWDGE DMAs of bf16 data ----
            wq_sb = cpool.tile([128, dt8, DL], BF16, tag="wq")
            nc.sync.dma_start(wq_sb[:], wq[:])
            wk_sb = cpool.tile([128, dt8, DL], BF16, tag="wk")
            nc.sync.dma_start(wk_sb[:], wk[:])
            wv_sb = cpool.tile([128, dt8, DL], BF16, tag="wv")
            nc.sync.dma_start(wv_sb[:], wv[:])
            cs128 = cpool.tile([128, s], BF16, tag="cs")
            nc.sync.dma_start(cs128[:], csd[:])
            sn128 = cpool.tile([128, s], BF16, tag="sn")
            nc.sync.dma_start(sn128[:], snd[:])
            mp_sb = cpool.tile([DL, DL], BF16, tag="mperm")
            nc.sync.dma_start(mp_sb[:], mperm[:])
            id_sb = cpool.tile([128, 128], BF16, tag="ident")
            nc.sync.dma_start(id_sb[:], identp[:])
            sel_sb = cpool.tile([H, N_CORES, 128], BF16, tag="sel")
            nc.sync.dma_start(sel_sb[:], selp[:])
            wo_sb = cpool.tile([128, dt8, D], BF16, tag="wo")
            biasc = cpool.tile([128, 1], F32, tag="biasc")
            nc.vector.memset(biasc[:], EXP_BIAS)
            # warmup collective: absorbs ncfw first-use latency and initial
            # cross-core skew while prep-b0 computes (no consumer)
            wu_in = dram.tile([N_CORES, 2, 16], BF16, tag="wu_in", name="wu_in")
            wu_out = dram.tile([N_CORES, 2, 16], BF16, tag="wu_out", name="wu_out")
            wu_src = cpool.tile([N_CORES, 2, 16], BF16, tag="wu_src")
            nc.vector.memset(wu_src[:], 0.0)
            nc.sync.dma_start(wu_in[:], wu_src[:])
            nc.gpsimd.collective_compute(
                "AllToAll",
                mybir.AluOpType.bypass,
                replica_groups=[list(range(N_CORES))],
                ins=[wu_in.opt()],
                outs=[wu_out.opt()],
            )
            # x^T tiles: per-chunk slots, rotated between batches. Batch-0
            # loads go on the scalar queue (idle before exp starts); batch-1
            # on sync.
            xt_t = {}
            for bi in range(b):
                for ch in range(nch):
                    xtile = xtpool.tile(
                        [128, dt8, chunk], BF16, tag=f"xtc{ch}", name="xtile"
                    )
                    eng = nc.scalar if bi == 0 else nc.sync
                    eng.dma_start(
                        xtile[:], xt[bi, ch].rearrange("(t p) q -> p t q", p=128)
                    )
                    xt_t[(bi, ch)] = xtile

            # A2A buffers: one pair per (batch, half)
            a2a_in = {}
            a2a_out = {}
            for bi in range(b):
                for hf in range(nhf):
                    t_in = dram.tile(
                        [N_CORES, HL * (HD + 1), rows_hf], BF16,
                        tag=f"a2a_in{bi}{hf}", name="a2a_in_t",
                    )
                    t_out = dram.tile(
                        [N_CORES, HL * (HD + 1), rows_hf], BF16,
                        tag=f"a2a_out{bi}{hf}", name="a2a_out_t",
                    )
                    a2a_in[(bi, hf)] = t_in
                    a2a_out[(bi, hf)] = t_out

            qkv_t = {}

            # ---------- prep units (QKV + RoPE + V) ----------
            def make_prep_units(bi):
                units = []
                for ch in range(nch):
                    def u_start(bi=bi, ch=ch):
                        if ch == 0:
                            qkv_t[bi] = (
                                qkvpool.tile([DL, s], BF16, tag="q_rope",
                                             name="q_rope"),
                                qkvpool.tile([DL, s], BF16, tag="k_rope",
                                             name="k_rope"),
                                qkvpool.tile([128, kt, HL, HD + 1], BF16,
                                             tag="v_sb", name="v_sb"),
                            )
                            nc.vector.memset(
                                qkv_t[bi][2][:, :, :, HD : HD + 1], 1.0
                            )

                    def u_proj(bi=bi, ch=ch, which=0):
                        xtile = xt_t[(bi, ch)]
                        cols = slice(ch * chunk, (ch + 1) * chunk)
                        wsb = (wq_sb, wk_sb, wv_sb)[which]
                        ps = psC.tile([128, chunk], F32, tag="tp", name="ps")
                        for dt in range(dt8):
                            nc.tensor.matmul(
                                ps[:],
                                wsb[:, dt, :],
                                xtile[:, dt, :],
                                start=(dt == 0),
                                stop=(dt == dt8 - 1),
                            )
                        if which == 2:
                            # v^T chunk -> transpose -> v_aug layout
                            vt_sb = ropepool.tile([DL, chunk], BF16, tag="vt",
                                                  name="vt")
                            nc.vector.tensor_copy(vt_sb[:], ps[:])
                            v_sb = qkv_t[bi][2]
                            vps = psC.tile([128, 4, 128], F32, tag="tp",
                                           name="vps")
                            for j in range(4):
                                nc.tensor.matmul(
                                    vps[:, j, :],
                                    vt_sb[:, j * 128 : (j + 1) * 128],
                                    id_sb[:],
                                    start=True,
                                    stop=True,
                                )
                            for h in range(HL):
                                nc.vector.tensor_copy(
                                    v_sb[:, ch * 4 : (ch + 1) * 4, h, 0:HD],
                                    vps[:, :, h * HD : (h + 1) * HD],
                                )
                        else:
                            dst = qkv_t[bi][which]
                            tsb = ropepool.tile([128, chunk], BF16, tag="tsb",
                                                name="tsb")
                            nc.vector.tensor_copy(tsb[:], ps[:])
                            rps = psC.tile([128, chunk], F32, tag="tp",
                                           name="rps")
                            nc.tensor.matmul(
                                rps[:], mp_sb[:], tsb[:], start=True, stop=True
                            )
                            m1 = ropepool.tile([128, chunk], BF16, tag="m1",
                                               name="m1")
                            nc.vector.tensor_tensor(
                                m1[:], ps[:], cs128[:, cols],
                                mybir.AluOpType.mult,
                            )
                            m2 = ropepool.tile([128, chunk], BF16, tag="m2",
                                               name="m2")
                            nc.vector.tensor_tensor(
                                m2[:], rps[:], sn128[:, cols],
                                mybir.AluOpType.mult,
                            )
                            nc.vector.tensor_tensor(
                                dst[:, cols], m1[:], m2[:], mybir.AluOpType.add
                            )

                    def u_q(bi=bi, ch=ch, _s=u_start, _p=u_proj):
                        _s()
                        _p(bi, ch, 0)

                    units.append(u_q)
                    units.append(lambda bi=bi, ch=ch, _p=u_proj: _p(bi, ch, 1))
                    units.append(lambda bi=bi, ch=ch, _p=u_proj: _p(bi, ch, 2))
                return units

            # ---------- attention ----------
            def emit_attn(bi, ch, work_q, feed_at):
                q_rope, k_rope, v_sb = qkv_t[bi]
                cols = slice(ch * chunk, (ch + 1) * chunk)
                hf = ch // 2
                cb = ch % 2  # column block within the A2A half payload
                pts = {}
                pv = {
                    h: psB.tile([HD + 1, chunk], F32, tag="pv", name="pv")
                    for h in range(HL)
                }

                def pv_group(gi):
                    ptp = pts[gi]
                    for h in range(HL):
                        for j in range(GMAX):
                            ktt = gi * GMAX + j
                            nc.tensor.matmul(
                                pv[h][:],
                                v_sb[:, ktt, h, :],
                                ptp[:, h, j, :],
                                start=(ktt == 0),
                                stop=(ktt == kt - 1),
                            )

                for gi in range(ngr):
                    sgp = psS.tile(
                        [128, HL, GMAX, chunk], F32, tag="sc", name="sgp"
                    )
                    for j in range(GMAX):
                        ktt = gi * GMAX + j
                        for h in range(HL):
                            rows = slice(h * HD, (h + 1) * HD)
                            nc.tensor.matmul(
                                sgp[:, h, j, :],
                                k_rope[rows, ktt * 128 : (ktt + 1) * 128],
                                q_rope[rows, cols],
                                start=True,
                                stop=True,
                            )
                    if gi > 0:
                        pv_group(gi - 1)
                    ptp = ptpool.tile(
                        [128, HL, GMAX, chunk], BF16, tag="pt", name="ptp"
                    )
                    # one wide exp covers both heads' kt-pair (4 banks)
                    nc.scalar.activation(
                        ptp[:],
                        sgp[:],
                        mybir.ActivationFunctionType.Exp,
                        bias=biasc[:],
                        scale=EXP_SCALE,
                    )
                    pts[gi] = ptp
                    for _ in range(feed_at.get(gi, 0)):
                        if work_q:
                            work_q.pop(0)()
                pv_group(ngr - 1)

                for h in range(HL):
                    oh = ohpool.tile([HD + 1, chunk], BF16, tag="oh", name="oh")
                    nc.vector.tensor_copy(oh[:], pv[h][:])
                    # numerator + sigma row (65 rows) for all 8 dst cores in
                    # one strided store into this half's A2A buffer
                    dst = a2a_in[(bi, hf)][
                        :, h * (HD + 1) : (h + 1) * (HD + 1),
                        cb * qb : (cb + 1) * qb,
                    ].rearrange("j p q -> p j q")
                    nc.gpsimd.dma_start(
                        dst, oh[:].rearrange("p (j q) -> p j q", j=N_CORES)
                    )

            def emit_a2a(bi, hf):
                nc.gpsimd.collective_compute(
                    "AllToAll",
                    mybir.AluOpType.bypass,
                    replica_groups=[list(range(N_CORES))],
                    ins=[a2a_in[(bi, hf)].opt()],
                    outs=[a2a_out[(bi, hf)].opt()],
                )

            # ---------- consumer (normalize + Wo) ----------
            def make_consumer_units(bi, hf):
                state = {}

                def c_recv():
                    recv = rcpool.tile(
                        [128, N_CORES, rows_hf], BF16, tag="recv", name="recv"
                    )
                    sigr = rcpool.tile([H, rows_hf], BF16, tag="sigr",
                                       name="sigr")
                    src = a2a_out[(bi, hf)]
                    for h in range(HL):
                        nc.sync.dma_start(
                            recv[h * HD : (h + 1) * HD, :, :],
                            src[
                                :, h * (HD + 1) : h * (HD + 1) + HD, :
                            ].rearrange("i p q -> p i q"),
                        )
                        nc.sync.dma_start(
                            sigr[h * N_CORES : (h + 1) * N_CORES, :],
                            src[:, h * (HD + 1) + HD, :],
                        )
                    rf = rcpool.tile([H, rows_hf], F32, tag="rf", name="rf")
                    nc.vector.reciprocal(rf[:], sigr[:])
                    rb = rcpool.tile([H, rows_hf], BF16, tag="rb", name="rb")
                    nc.vector.tensor_copy(rb[:], rf[:])
                    state["recv"] = recv
                    state["rb"] = rb

                def c_scale():
                    recv, rb = state["recv"], state["rb"]
                    for i in range(N_CORES):
                        bc = psC.tile([128, rows_hf], F32, tag="tp", name="bc")
                        nc.tensor.matmul(
                            bc[:], sel_sb[:, i, :], rb[:], start=True, stop=True
                        )
                        nc.vector.tensor_tensor(
                            recv[:, i, :], recv[:, i, :], bc[:],
                            mybir.AluOpType.mult,
                        )

                def c_wo(nco):
                    recv = state["recv"]
                    wps = psC.tile([128, 512], F32, tag="tp", name="wps")
                    for i in range(N_CORES):
                        nc.tensor.matmul(
                            wps[:],
                            recv[:, i, :],
                            wo_sb[:, i, nco * 512 : (nco + 1) * 512],
                            start=(i == 0),
                            stop=(i == N_CORES - 1),
                        )
                    osb = outpool.tile([128, 512], F32, tag="osb", name="osb")
                    nc.vector.tensor_copy(osb[:], wps[:])
                    r0 = bi * (shard // b) + hf * rows_hf
                    nc.sync.dma_start(
                        out[r0 : r0 + rows_hf, nco * 512 : (nco + 1) * 512],
                        osb[:],
                    )

                def c_all():
                    c_recv()
                    c_scale()
                    c_wo(0)
                    c_wo(1)

                return [c_all]

            # ---------- main flow ----------
            wo_loaded = []

            def load_wo():
                if not wo_loaded:
                    nc.sync.dma_start(wo_sb[:], wo[:])
                    wo_loaded.append(True)

            prep0 = make_prep_units(0)
            for u in prep0:
                u()

            prep1 = list(make_prep_units(1))
            # weave batch-1 prep into batch-0 attention: 12 units over 4
            # chunks -> 3 units per chunk at groups 2, 4, 6
            feed_prep = {2: 1, 4: 1, 6: 1}
            emit_attn(0, 0, prep1, feed_prep)
            emit_attn(0, 1, prep1, feed_prep)
            emit_a2a(0, 0)
            emit_attn(0, 2, prep1, feed_prep)
            emit_attn(0, 3, prep1, feed_prep)
            load_wo()
            emit_a2a(0, 1)
            # consumer blocks under simulated-time floors so the scheduler
            # cannot hoist them ahead of independent attention work (their
            # A2A dependency lands later than the cost model thinks on the
            # first collective / under cross-core skew)
            with tc.tile_wait_until(0.140):
                for u in make_consumer_units(0, 0):
                    u()
            emit_attn(1, 0, [], {})
            with tc.tile_wait_until(0.175):
                for u in make_consumer_units(0, 1):
                    u()
            emit_attn(1, 1, [], {})
            emit_a2a(1, 0)
            emit_attn(1, 2, [], {})
            with tc.tile_wait_until(0.220):
                for u in make_consumer_units(1, 0):
                    u()
            emit_attn(1, 3, [], {})
            emit_a2a(1, 1)
            with tc.tile_wait_until(0.260):
                for u in make_consumer_units(1, 1):
                    u()

    split_excess_waits(nc)
    return nc


def make_in_maps(x, cos, sin, Wq, Wk, Wv, Wo, b, s):
    nch = s // 512
    x = np.asarray(x, dtype=np.float32)
    # x^T per (batch, chunk): [b, nch, D, 512] bf16, contiguous
    xt = np.ascontiguousarray(
        x.reshape(b, nch, 512, D).transpose(0, 1, 3, 2)
    ).astype(BF16_NP)
    csd = np.ascontiguousarray(np.tile(np.asarray(cos).T, (4, 1))).astype(BF16_NP)
    snd = np.ascontiguousarray(np.tile(np.asarray(sin).T, (4, 1))).astype(BF16_NP)
    wo_m = np.ascontiguousarray(
        np.asarray(Wo, dtype=np.float32).reshape(8, 128, D).transpose(1, 0, 2)
    ).astype(BF16_NP)
    mperm = _perm_matrix().astype(BF16_NP)
    ident = np.eye(128, dtype=np.float32).astype(BF16_NP)
    sel = _sel_matrix().astype(BF16_NP)
    in_maps = []
    for c in range(N_CORES):
        cs = slice(c * DL, (c + 1) * DL)
        def wslice(W):
            ws = np.asarray(W, dtype=np.float32)[:, cs]
            return np.ascontiguousarray(
                ws.reshape(8, 128, DL).transpose(1, 0, 2)
            ).astype(BF16_NP)
        in_maps.append(
            {
                "xt": xt,
                "csd": csd,
                "snd": snd,
                "wq": wslice(Wq),
                "wk": wslice(Wk),
                "wv": wslice(Wv),
                "wo": wo_m,
                "mperm": mperm,
                "ident": ident,
                "sel": sel,
            }
        )
    return in_maps


_NC_CACHE = {}


def run(x, cos, sin, Wq, Wk, Wv, Wo, trace=False, chunk=512):
    b, s, _ = x.shape
    key = (b, s, chunk)
    if key not in _NC_CACHE:
        _NC_CACHE[key] = build_nc(b=b, s=s, chunk=chunk)
    nc = _NC_CACHE[key]
    in_maps = make_in_maps(x, cos, sin, Wq, Wk, Wv, Wo, b, s)
    res = run_bass_kernel_spmd(nc, in_maps, list(range(N_CORES)), trace=trace)
    # unshard: core c's out rows [bi*256 + hf*128 + (0..127)] map to
    # full[bi, (2*hf + (r>=64))*512 + c*64 + r%64]
    full = np.empty((b, s, D), dtype=np.float32)
    for c in range(N_CORES):
        o = res.results[c]["out"]
        for bi in range(b):
            for hf in range(2):
                blk = o[bi * 256 + hf * 128 : bi * 256 + (hf + 1) * 128]
                q0 = (2 * hf) * 512 + c * 64
                q1 = (2 * hf + 1) * 512 + c * 64
                full[bi, q0 : q0 + 64] = blk[0:64]
                full[bi, q1 : q1 + 64] = blk[64:128]
    return full, res


def kernel(x, cos, sin, Wq, Wk, Wv, Wo):
    out, _ = run(
        np.asarray(x), np.asarray(cos), np.asarray(sin),
        np.asarray(Wq), np.asarray(Wk), np.asarray(Wv), np.asarray(Wo),
    )
    return out.astype(np.float32)


# revision 16
# speedup vs baseline: 1.0286x; 1.0286x over previous
"""Multi-head attention with RoPE on 8 Trainium2 NeuronCores.

Problem: x[2,2048,1024] -> MHA(16 heads, hd=64, NeoX RoPE, non-causal) -> out.

Sharding: tensor-parallel over heads (each core owns 2 heads over the full
sequence). The output is resharded so each core owns 64-row blocks of every
512-row q-chunk of both batches (strided blocks), which lets the AllToAll
run in four quarter-sized pieces that overlap compute.

Host-side marshalling (inside kernel(), plain numpy): weights column-sliced
per core, x transposed per (batch, chunk) and cast to bf16, cos/sin
transposed+tiled — so the device kernel does no fp32 casting, no x
transposes, and loads everything with fast HWDGE DMAs.

Device pipeline (per core):
  - QKV projections + NeoX RoPE from the pre-transposed x^T,
  - flash-style attention with transposed scores [s_k, s_q]; the two local
    heads' scores land in one 4-bank PSUM group and one wide exp instruction
    per group covers both heads; softmax denominator from a fused ones
    column in V (exp bias keeps the fp32 range safe),
  - unnormalized numerator + sigma rows stored straight into half-batch
    AllToAll buffers (strided per-core 64-row blocks); each of the four
    A2As + normalization + Wo overlaps later attention chunks,
  - consumer-side normalization: 1/sigma partition-broadcast via a K=16
    selection-matrix matmul, multiplied into recv straight from PSUM,
  - emission order weaves next-batch prep and consumer work into the
    attention group loop so the PE instruction FIFO always has work.

All matmuls bf16 (fp32 PSUM); rel-err tolerance is 2e-2.
"""

import sys

sys.path.insert(0, "/opt/trn_rl_repo")

import ml_dtypes  # noqa: E402
import numpy as np  # noqa: E402

import concourse.bass as bass  # noqa: E402
import concourse.mybir as mybir  # noqa: E402
import concourse.tile as tile  # noqa: E402
from concourse.bass_utils import run_bass_kernel_spmd  # noqa: E402

N_CORES = 8
D = 1024
H = 16
HD = 64
HL = H // N_CORES  # 2 local heads per core
DL = HL * HD  # 128 local attn dims
EXP_SCALE = 0.125  # 1/sqrt(hd)
EXP_BIAS = -24.0  # exp(s/8 - 24): cancels in softmax, keeps fp32 range safe
GMAX = 2  # kt tiles per score group; one exp instruction covers both heads

F32 = mybir.dt.float32
BF16 = mybir.dt.bfloat16
BF16_NP = ml_dtypes.bfloat16


def _perm_matrix():
    """lhsT for the rotate_half matmul: qrot^T = lhsT.T @ q^T."""
    mt = np.zeros((DL, DL), dtype=np.float32)
    for o in (0, HD):
        for r in range(HD // 2):
            mt[o + r, o + r + HD // 2] = -1.0
            mt[o + r + HD // 2, o + r] = 1.0
    return np.ascontiguousarray(mt.T)


def _sel_matrix():
    """lhsT blocks for the 1/sigma partition-broadcast matmul.

    sigr partition layout is h*8+i (h = local head, i = source core);
    sel[r, i, p] = 1 iff r == (p//64)*8 + i.
    """
    sel = np.zeros((H, N_CORES, 128), dtype=np.float32)
    for i in range(N_CORES):
        sel[0 * N_CORES + i, i, 0:HD] = 1.0
        sel[1 * N_CORES + i, i, HD:128] = 1.0
    return sel


def split_excess_waits(nc, max_waits=1):
    """This container's walrus rejects >1 semaphore wait per instruction;
    split excess waits onto NoOp carriers on the same engine."""
    for bb in nc.m.functions[0].blocks:
        insts = bb.instructions
        idx = 0
        while idx < len(insts):
            ins = insts[idx]
            si = ins.sync_info
            if si is not None and si.on_wait and len(si.on_wait) > max_waits:
                ow = list(si.on_wait)
                si.on_wait = ow[-max_waits:]
                extra = ow[:-max_waits]
                k = 0
                while extra:
                    chunk, extra = extra[:max_waits], extra[max_waits:]
                    c = mybir.InstNoOp(name=f"{ins.name}-ws{k}", ins=[], outs=[])
                    c.engine = ins.engine
                    c.sync_info = mybir.SyncInfo(on_wait=chunk, on_update=[])
                    nc.register_instruction(c)
                    insts.insert(idx, c)
                    idx += 1
                    k += 1
            idx += 1


def build_nc(b=2, s=2048, chunk=512):
    kt = s // 128  # 16 score tiles along s_k per batch
    nch = s // chunk  # 4 s_q chunks per batch
    nhf = nch // 2  # 2 A2A halves per batch
    qb = chunk // N_CORES  # 64-row block per core per chunk
    rows_hf = 2 * qb  # 128 rows per core per A2A half
    shard = b * nch * qb  # 512 output rows per core
    ngr = kt // GMAX  # 8 score groups per chunk
    dt8 = D // 128
    assert nch == 4 and ngr == 8 and qb == 64

    nc = bass.Bass()
    xt = nc.declare_dram_parameter("xt", [b, nch, D, chunk], BF16, isOutput=False)
    csd = nc.declare_dram_parameter("csd", [128, s], BF16, isOutput=False)
    snd = nc.declare_dram_parameter("snd", [128, s], BF16, isOutput=False)
    wq = nc.declare_dram_parameter("wq", [128, dt8, DL], BF16, isOutput=False)
    wk = nc.declare_dram_parameter("wk", [128, dt8, DL], BF16, isOutput=False)
    wv = nc.declare_dram_parameter("wv", [128, dt8, DL], BF16, isOutput=False)
    wo = nc.declare_dram_parameter("wo", [128, dt8, D], BF16, isOutput=False)
    mperm = nc.declare_dram_parameter("mperm", [DL, DL], BF16, isOutput=False)
    identp = nc.declare_dram_parameter("ident", [128, 128], BF16, isOutput=False)
    selp = nc.declare_dram_parameter("sel", [H, N_CORES, 128], BF16, isOutput=False)
    out = nc.declare_dram_parameter("out", [shard, D], F32, isOutput=True)

    with tile.TileContext(nc) as tc:
        with (
            tc.tile_pool(name="dram", bufs=1, space="DRAM") as dram,
            tc.tile_pool(name="const", bufs=1) as cpool,
            tc.tile_pool(name="xt", bufs=1) as xtpool,
            tc.tile_pool(name="qkv", bufs=2) as qkvpool,
            tc.tile_pool(name="rope", bufs=2) as ropepool,
            tc.tile_pool(name="pt", bufs=4) as ptpool,
            tc.tile_pool(name="oh", bufs=4) as ohpool,
            tc.tile_pool(name="rc", bufs=2) as rcpool,
            tc.tile_pool(name="outp", bufs=2) as outpool,
            # PSUM (8 banks): "sc" score group [128, HL, GMAX, 512] fp32 = 4
            # banks (single-buffered; one wide exp per group drains it).
            # "pv" 2 x [65, 512] = 2 banks. "tp" general purpose = 2 banks.
            tc.tile_pool(name="psS", bufs=1, space="PSUM") as psS,
            tc.tile_pool(name="psB", bufs=2, space="PSUM") as psB,
            tc.tile_pool(name="psC", bufs=2, space="PSUM") as psC,
        ):
            # ---- staging: all loads are plain HWDGE DMAs of bf16 data ----
            wq_sb = cpool.tile([128, dt8, DL], BF16, tag="wq")
            nc.sync.dma_start(wq_sb[:], wq[:])
            wk_sb = cpool.tile([128, dt8, DL], BF16, tag="wk")
            nc.sync.dma_start(wk_sb[:], wk[:])
            wv_sb = cpool.tile([128, dt8, DL], BF16, tag="wv")
            nc.sync.dma_start(wv_sb[:], wv[:])
            cs128 = cpool.tile([128, s], BF16, tag="cs")
            nc.sync.dma_start(cs128[:], csd[:])
            sn128 = cpool.tile([128, s], BF16, tag="sn")
            nc.sync.dma_start(sn128[:], snd[:])
            mp_sb = cpool.tile([DL, DL], BF16, tag="mperm")
            nc.sync.dma_start(mp_sb[:], mperm[:])
            id_sb = cpool.tile([128, 128], BF16, tag="ident")
            nc.sync.dma_start(id_sb[:], identp[:])
            sel_sb = cpool.tile([H, N_CORES, 128], BF16, tag="sel")
            nc.sync.dma_start(sel_sb[:], selp[:])
            wo_sb = cpool.tile([128, dt8, D], BF16, tag="wo")
            biasc = cpool.tile([128, 1], F32, tag="biasc")
            nc.vector.memset(biasc[:], EXP_BIAS)
            # warmup collective: absorbs ncfw first-use latency and initial
            # cross-core skew while prep-b0 computes (no consumer)
            wu_in = dram.tile([N_CORES, 2, 16], BF16, tag="wu_in", name="wu_in")
            wu_out = dram.tile([N_CORES, 2, 16], BF16, tag="wu_out", name="wu_out")
            wu_src = cpool.tile([N_CORES, 2, 16], BF16, tag="wu_src")
            nc.vector.memset(wu_src[:], 0.0)
            nc.sync.dma_start(wu_in[:], wu_src[:])
            nc.gpsimd.collective_compute(
                "AllToAll",
                mybir.AluOpType.bypass,
                replica_groups=[list(range(N_CORES))],
                ins=[wu_in.opt()],
                outs=[wu_out.opt()],
            )
            # x^T tiles: per-chunk slots, rotated between batches. Batch-0
            # loads go on the scalar queue (idle before exp starts); batch-1
            # on sync.
            xt_t = {}
            for bi in range(b):
                for ch in range(nch):
                    xtile = xtpool.tile(
                        [128, dt8, chunk], BF16, tag=f"xtc{ch}", name="xtile"
                    )
                    eng = nc.scalar if bi == 0 else nc.sync
                    eng.dma_start(
                        xtile[:], xt[bi, ch].rearrange("(t p) q -> p t q", p=128)
                    )
                    xt_t[(bi, ch)] = xtile

            # A2A buffers: one pair per (batch, half)
            a2a_in = {}
            a2a_out = {}
            for bi in range(b):
                for hf in range(nhf):
                    t_in = dram.tile(
                        [N_CORES, HL * (HD + 1), rows_hf], BF16,
                        tag=f"a2a_in{bi}{hf}", name="a2a_in_t",
                    )
                    t_out = dram.tile(
                        [N_CORES, HL * (HD + 1), rows_hf], BF16,
                        tag=f"a2a_out{bi}{hf}", name="a2a_out_t",
                    )
                    a2a_in[(bi, hf)] = t_in
                    a2a_out[(bi, hf)] = t_out

            qkv_t = {}

            # ---------- prep units (QKV + RoPE + V) ----------
            def make_prep_units(bi):
                units = []
                for ch in range(nch):
                    def u_start(bi=bi, ch=ch):
                        if ch == 0:
                            qkv_t[bi] = (
                                qkvpool.tile([DL, s], BF16, tag="q_rope",
                                             name="q_rope"),
                                qkvpool.tile([DL, s], BF16, tag="k_rope",
                                             name="k_rope"),
                                qkvpool.tile([128, kt, HL, HD + 1], BF16,
                                             tag="v_sb", name="v_sb"),
                            )
                            nc.vector.memset(
                                qkv_t[bi][2][:, :, :, HD : HD + 1], 1.0
                            )

                    def u_proj(bi=bi, ch=ch, which=0):
                        xtile = xt_t[(bi, ch)]
                        cols = slice(ch * chunk, (ch + 1) * chunk)
                        wsb = (wq_sb, wk_sb, wv_sb)[which]
                        ps = psC.tile([128, chunk], F32, tag="tp", name="ps")
                        for dt in range(dt8):
                            nc.tensor.matmul(
                                ps[:],
                                wsb[:, dt, :],
                                xtile[:, dt, :],
                                start=(dt == 0),
                                stop=(dt == dt8 - 1),
                            )
                        if which == 2:
                            # v^T chunk -> transpose -> v_aug layout
                            vt_sb = ropepool.tile([DL, chunk], BF16, tag="vt",
                                                  name="vt")
                            nc.vector.tensor_copy(vt_sb[:], ps[:])
                            v_sb = qkv_t[bi][2]
                            vps = psC.tile([128, 4, 128], F32, tag="tp",
                                           name="vps")
                            for j in range(4):
                                nc.tensor.matmul(
                                    vps[:, j, :],
                                    vt_sb[:, j * 128 : (j + 1) * 128],
                                    id_sb[:],
                                    start=True,
                                    stop=True,
                                )
                            for h in range(HL):
                                nc.vector.tensor_copy(
                                    v_sb[:, ch * 4 : (ch + 1) * 4, h, 0:HD],
                                    vps[:, :, h * HD : (h + 1) * HD],
                                )
                        else:
                            dst = qkv_t[bi][which]
                            tsb = ropepool.tile([128, chunk], BF16, tag="tsb",
                                                name="tsb")
                            nc.vector.tensor_copy(tsb[:], ps[:])
                            rps = psC.tile([128, chunk], F32, tag="tp",
                                           name="rps")
                            nc.tensor.matmul(
                                rps[:], mp_sb[:], tsb[:], start=True, stop=True
                            )
                            m1 = ropepool.tile([128, chunk], BF16, tag="m1",
                                               name="m1")
                            nc.vector.tensor_tensor(
                                m1[:], ps[:], cs128[:, cols],
                                mybir.AluOpType.mult,
                            )
                            m2 = ropepool.tile([128, chunk], BF16, tag="m2",
                                               name="m2")
                            nc.vector.tensor_tensor(
                                m2[:], rps[:], sn128[:, cols],
                                mybir.AluOpType.mult,
                            )
                            nc.vector.tensor_tensor(
                                dst[:, cols], m1[:], m2[:], mybir.AluOpType.add
                            )

                    def u_q(bi=bi, ch=ch, _s=u_start, _p=u_proj):
                        _s()
                        _p(bi, ch, 0)

                    units.append(u_q)
                    units.append(lambda bi=bi, ch=ch, _p=u_proj: _p(bi, ch, 1))
                    units.append(lambda bi=bi, ch=ch, _p=u_proj: _p(bi, ch, 2))
                return units

            # ---------- attention ----------
            def emit_attn(bi, ch, work_q, feed_at):
                q_rope, k_rope, v_sb = qkv_t[bi]
                cols = slice(ch * chunk, (ch + 1) * chunk)
                hf = ch // 2
                cb = ch % 2  # column block within the A2A half payload
                pts = {}
                pv = {
                    h: psB.tile([HD + 1, chunk], F32, tag="pv", name="pv")
                    for h in range(HL)
                }

                def pv_group(gi):
                    ptp = pts[gi]
                    for h in range(HL):
                        for j in range(GMAX):
                            ktt = gi * GMAX + j
                            nc.tensor.matmul(
                                pv[h][:],
                                v_sb[:, ktt, h, :],
                                ptp[:, h, j, :],
                                start=(ktt == 0),
                                stop=(ktt == kt - 1),
                            )

                for gi in range(ngr):
                    sgp = psS.tile(
                        [128, HL, GMAX, chunk], F32, tag="sc", name="sgp"
                    )
                    for j in range(GMAX):
                        ktt = gi * GMAX + j
                        for h in range(HL):
                            rows = slice(h * HD, (h + 1) * HD)
                            nc.tensor.matmul(
                                sgp[:, h, j, :],
                                k_rope[rows, ktt * 128 : (ktt + 1) * 128],
                                q_rope[rows, cols],
                                start=True,
                                stop=True,
                            )
                    if gi > 0:
                        pv_group(gi - 1)
                    ptp = ptpool.tile(
                        [128, HL, GMAX, chunk], BF16, tag="pt", name="ptp"
                    )
                    # one wide exp covers both heads' kt-pair (4 banks)
                    nc.scalar.activation(
                        ptp[:],
                        sgp[:],
                        mybir.ActivationFunctionType.Exp,
                        bias=biasc[:],
                        scale=EXP_SCALE,
                    )
                    pts[gi] = ptp
                    for _ in range(feed_at.get(gi, 0)):
                        if work_q:
                            work_q.pop(0)()
                pv_group(ngr - 1)

                for h in range(HL):
                    oh = ohpool.tile([HD + 1, chunk], BF16, tag="oh", name="oh")
                    nc.vector.tensor_copy(oh[:], pv[h][:])
                    # numerator + sigma row (65 rows) for all 8 dst cores in
                    # one strided store into this half's A2A buffer
                    dst = a2a_in[(bi, hf)][
                        :, h * (HD + 1) : (h + 1) * (HD + 1),
                        cb * qb : (cb + 1) * qb,
                    ].rearrange("j p q -> p j q")
                    nc.gpsimd.dma_start(
                        dst, oh[:].rearrange("p (j q) -> p j q", j=N_CORES)
                    )

            def emit_a2a(bi, hf):
                nc.gpsimd.collective_compute(
                    "AllToAll",
                    mybir.AluOpType.bypass,
                    replica_groups=[list(range(N_CORES))],
                    ins=[a2a_in[(bi, hf)].opt()],
                    outs=[a2a_out[(bi, hf)].opt()],
                )

            # ---------- consumer (normalize + Wo) ----------
            def make_consumer_units(bi, hf, state=None):
                if state is None:
                    state = {}

                def c_recv():
                    recv = rcpool.tile(
                        [128, N_CORES, rows_hf], BF16, tag="recv", name="recv"
                    )
                    sigr = rcpool.tile([H, rows_hf], BF16, tag="sigr",
                                       name="sigr")
                    src = a2a_out[(bi, hf)]
                    for h in range(HL):
                        nc.sync.dma_start(
                            recv[h * HD : (h + 1) * HD, :, :],
                            src[
                                :, h * (HD + 1) : h * (HD + 1) + HD, :
                            ].rearrange("i p q -> p i q"),
                        )
                        nc.sync.dma_start(
                            sigr[h * N_CORES : (h + 1) * N_CORES, :],
                            src[:, h * (HD + 1) + HD, :],
                        )
                    rf = rcpool.tile([H, rows_hf], F32, tag="rf", name="rf")
                    nc.vector.reciprocal(rf[:], sigr[:])
                    rb = rcpool.tile([H, rows_hf], BF16, tag="rb", name="rb")
                    nc.vector.tensor_copy(rb[:], rf[:])
                    state["recv"] = recv
                    state["rb"] = rb

                def c_scale():
                    recv, rb = state["recv"], state["rb"]
                    for i in range(N_CORES):
                        bc = psC.tile([128, rows_hf], F32, tag="tp", name="bc")
                        nc.tensor.matmul(
                            bc[:], sel_sb[:, i, :], rb[:], start=True, stop=True
                        )
                        nc.vector.tensor_tensor(
                            recv[:, i, :], recv[:, i, :], bc[:],
                            mybir.AluOpType.mult,
                        )

                def c_wo(nco):
                    recv = state["recv"]
                    wps = psC.tile([128, 512], F32, tag="tp", name="wps")
                    for i in range(N_CORES):
                        nc.tensor.matmul(
                            wps[:],
                            recv[:, i, :],
                            wo_sb[:, i, nco * 512 : (nco + 1) * 512],
                            start=(i == 0),
                            stop=(i == N_CORES - 1),
                        )
                    osb = outpool.tile([128, 512], F32, tag="osb", name="osb")
                    nc.vector.tensor_copy(osb[:], wps[:])
                    r0 = bi * (shard // b) + hf * rows_hf
                    nc.sync.dma_start(
                        out[r0 : r0 + rows_hf, nco * 512 : (nco + 1) * 512],
                        osb[:],
                    )

                def c_all():
                    c_recv()
                    c_scale()
                    c_wo(0)
                    c_wo(1)

                return [c_all]

            # ---------- main flow ----------
            wo_loaded = []

            def load_wo():
                if not wo_loaded:
                    nc.sync.dma_start(wo_sb[:], wo[:])
                    wo_loaded.append(True)

            prep0 = make_prep_units(0)
            for u in prep0:
                u()

            prep1 = list(make_prep_units(1))
            # weave batch-1 prep into batch-0 attention: 12 units over 4
            # chunks -> 3 units per chunk at groups 2, 4, 6
            feed_prep = {2: 1, 4: 1, 6: 1}
            emit_attn(0, 0, prep1, feed_prep)
            emit_attn(0, 1, prep1, feed_prep)
            emit_a2a(0, 0)
            emit_attn(0, 2, prep1, feed_prep)
            emit_attn(0, 3, prep1, feed_prep)
            load_wo()
            emit_a2a(0, 1)
            # consumer blocks under simulated-time floors so the scheduler
            # cannot hoist them ahead of independent attention work (their
            # A2A dependency lands later than the cost model thinks on the
            # first collective / under cross-core skew)
            with tc.tile_wait_until(0.140):
                for u in make_consumer_units(0, 0):
                    u()
            emit_attn(1, 0, [], {})
            with tc.tile_wait_until(0.175):
                for u in make_consumer_units(0, 1):
                    u()
            emit_attn(1, 1, [], {})
            emit_a2a(1, 0)
            emit_attn(1, 2, [], {})
            cons10_state = {}
            with tc.tile_wait_until(0.220):
                for u in make_consumer_units(1, 0, cons10_state):
                    u()
            # alignment gate: all cores start attn(1,3) only once A2A(1,0)
            # has landed. The straggler reaches this point naturally, so it
            # loses nothing; fast cores absorb the skew here (overlapped
            # with their consumer work) instead of idling at the final A2A.
            recv10 = cons10_state["recv"]
            gz = rcpool.tile([1, 16], BF16, tag="gz", name="gz")
            nc.vector.tensor_tensor(
                gz[:], recv10[0:1, 0, 0:16], recv10[0:1, 0, 0:16],
                mybir.AluOpType.subtract,
            )
            kr1 = qkv_t[1][1]
            nc.vector.tensor_tensor(
                kr1[0:1, 0:16], kr1[0:1, 0:16], gz[:], mybir.AluOpType.add
            )
            emit_attn(1, 3, [], {})
            emit_a2a(1, 1)
            with tc.tile_wait_until(0.260):
                for u in make_consumer_units(1, 1):
                    u()

    split_excess_waits(nc)
    return nc


def make_in_maps(x, cos, sin, Wq, Wk, Wv, Wo, b, s):
    nch = s // 512
    x = np.asarray(x, dtype=np.float32)
    # x^T per (batch, chunk): [b, nch, D, 512] bf16, contiguous
    xt = np.ascontiguousarray(
        x.reshape(b, nch, 512, D).transpose(0, 1, 3, 2)
    ).astype(BF16_NP)
    csd = np.ascontiguousarray(np.tile(np.asarray(cos).T, (4, 1))).astype(BF16_NP)
    snd = np.ascontiguousarray(np.tile(np.asarray(sin).T, (4, 1))).astype(BF16_NP)
    wo_m = np.ascontiguousarray(
        np.asarray(Wo, dtype=np.float32).reshape(8, 128, D).transpose(1, 0, 2)
    ).astype(BF16_NP)
    mperm = _perm_matrix().astype(BF16_NP)
    ident = np.eye(128, dtype=np.float32).astype(BF16_NP)
    sel = _sel_matrix().astype(BF16_NP)
    in_maps = []
    for c in range(N_CORES):
        cs = slice(c * DL, (c + 1) * DL)
        def wslice(W):
            ws = np.asarray(W, dtype=np.float32)[:, cs]
            return np.ascontiguousarray(
                ws.reshape(8, 128, DL).transpose(1, 0, 2)
            ).astype(BF16_NP)
        in_maps.append(
            {
                "xt": xt,
                "csd": csd,
                "snd": snd,
                "wq": wslice(Wq),
                "wk": wslice(Wk),
                "wv": wslice(Wv),
                "wo": wo_m,
                "mperm": mperm,
                "ident": ident,
                "sel": sel,
            }
        )
    return in_maps


_NC_CACHE = {}


def run(x, cos, sin, Wq, Wk, Wv, Wo, trace=False, chunk=512):
    b, s, _ = x.shape
    key = (b, s, chunk)
    if key not in _NC_CACHE:
        _NC_CACHE[key] = build_nc(b=b, s=s, chunk=chunk)
    nc = _NC_CACHE[key]
    in_maps = make_in_maps(x, cos, sin, Wq, Wk, Wv, Wo, b, s)
    res = run_bass_kernel_spmd(nc, in_maps, list(range(N_CORES)), trace=trace)
    # unshard: core c's out rows [bi*256 + hf*128 + (0..127)] map to
    # full[bi, (2*hf + (r>=64))*512 + c*64 + r%64]
    full = np.empty((b, s, D), dtype=np.float32)
    for c in range(N_CORES):
        o = res.results[c]["out"]
        for bi in range(b):
            for hf in range(2):
                blk = o[bi * 256 + hf * 128 : bi * 256 + (hf + 1) * 128]
                q0 = (2 * hf) * 512 + c * 64
                q1 = (2 * hf + 1) * 512 + c * 64
                full[bi, q0 : q0 + 64] = blk[0:64]
                full[bi, q1 : q1 + 64] = blk[64:128]
    return full, res


def kernel(x, cos, sin, Wq, Wk, Wv, Wo):
    out, _ = run(
        np.asarray(x), np.asarray(cos), np.asarray(sin),
        np.asarray(Wq), np.asarray(Wk), np.asarray(Wv), np.asarray(Wo),
    )
    return out.astype(np.float32)
